# revision 19
# baseline (speedup 1.0000x reference)
"""MultiHeadAttention forward (B=8, S=1024, DM=1024, H=16) on 8 trn2 cores.

Sharding: data-parallel over batch. Each core runs an identical Bass/Tile
program on one batch element and produces its (S, DM) output slice plus its
(H, S, S) attention-weights slice.

Returns (output, weights) matching the reference tuple.
"""

import os

import numpy as np

import concourse.bass as bass
import concourse.mybir as mybir
from concourse import bacc
from concourse.bass_utils import run_bass_kernel_spmd
from concourse.tile import TileContext

S = 1024
DM = 1024
H = 16
DH = 64
P = 128
NT = DM // P  # 8 tiles of 128
B = 8
HALF = 512  # fp32 moving-operand / psum-bank max free dim

F32 = mybir.dt.float32
# float32r runs the PE at 1 cycle/row (vs 4 for plain fp32) when N >= 256.
USE_F32R = os.environ.get("KERNEL_F32", "0") != "1"
MM_DT = mybir.dt.float32r if USE_F32R else F32

AOP = mybir.AluOpType
ACT_EXP = mybir.ActivationFunctionType.Exp


def _bcast(ap_1d, nparts):
    """Broadcast a flat DRAM AP across `nparts` partitions (step-0 partition dim)."""
    return bass.AP(
        tensor=ap_1d.tensor, offset=ap_1d.offset, ap=[[0, nparts]] + list(ap_1d.ap)
    )


def build_nc():
    nc = bacc.Bacc()

    q_d = nc.dram_tensor("Qb", [S, DM], F32, kind="ExternalInput")
    k_d = nc.dram_tensor("Kb", [S, DM], F32, kind="ExternalInput")
    v_d = nc.dram_tensor("Vb", [S, DM], F32, kind="ExternalInput")
    wq_d = nc.dram_tensor("Wq", [DM, DM], MM_DT, kind="ExternalInput")
    bq_d = nc.dram_tensor("bq", [DM], F32, kind="ExternalInput")
    wk_d = nc.dram_tensor("Wk", [DM, DM], MM_DT, kind="ExternalInput")
    bk_d = nc.dram_tensor("bk", [DM], F32, kind="ExternalInput")
    wv_d = nc.dram_tensor("Wv", [DM, DM], MM_DT, kind="ExternalInput")
    bv_d = nc.dram_tensor("bv", [DM], F32, kind="ExternalInput")
    wo_d = nc.dram_tensor("Wo", [DM, DM], MM_DT, kind="ExternalInput")
    bo_d = nc.dram_tensor("bo", [DM], F32, kind="ExternalInput")

    id_d = nc.dram_tensor("ident", [P, P], F32, kind="ExternalInput")
    out_d = nc.dram_tensor("out", [S, DM], F32, kind="ExternalOutput")
    wts_d = nc.dram_tensor("wts", [H, S, S], F32, kind="ExternalOutput")

    with TileContext(nc) as tc:
        with (
            tc.tile_pool(name="const", bufs=1) as const,
            tc.tile_pool(name="qkv", bufs=1) as qkv,
        ):
            def warm_psum(pool, shape, tag, n):
                # First toucher of freshly (re)allocated PSUM banks must not
                # be a HW-decoded PE matmul (1-wait budget): cycle each slot
                # through a DVE memset so pool-boundary waits land on DVE.
                for b in range(n):
                    t = pool.tile(shape, F32, name=f"warm_{pool.name}_{tag}{b}", tag=tag)
                    nc.vector.memset(t, 0.0)

            def claim(tile_ap):
                # In-place DVE copy: makes DVE the tile's last writer so PE
                # consumers wait a (coalescable) DVE tick, not a HWDGE sem
                # (self-loading f32/f32r matmuls lower to an LDWEIGHTS with a
                # 1-semaphore-wait budget in walrus codegen).
                nc.vector.tensor_copy(tile_ap, tile_ap)
                return tile_ap

            ident = const.tile([P, P], F32, name="ident")
            nc.sync.dma_start(out=ident, in_=id_d[:, :])
            claim(ident)

            bqt = const.tile([P, NT], F32, name="bqt")
            nc.sync.dma_start(out=bqt, in_=bq_d[:].rearrange("(t p) -> p t", p=P))
            bkt = const.tile([P, NT], F32, name="bkt")
            nc.sync.dma_start(out=bkt, in_=bk_d[:].rearrange("(t p) -> p t", p=P))
            bv_bc = const.tile([P, DM], F32, name="bv_bc")
            nc.sync.dma_start(out=bv_bc, in_=_bcast(bv_d[:], P))
            bo_bc = const.tile([P, DM], F32, name="bo_bc")
            nc.sync.dma_start(out=bo_bc, in_=_bcast(bo_d[:], P))

            def load_transposed(x_d, nm, xpool):
                """DRAM [S, DM] -> list of NT SBUF tiles XT[dt] = X.T[dt*128:, :] ([dm, S])."""
                xt = [
                    xpool.tile([P, S], MM_DT, name=f"{nm}T{d}", tag=f"{nm}T{d}")
                    for d in range(NT)
                ]
                with (
                    tc.tile_pool(name=f"stg_{nm}", bufs=3) as stg,
                    tc.tile_pool(name=f"ptr_{nm}", bufs=4, space="PSUM") as ptr,
                ):
                    warm_psum(ptr, [P, P], "pt", 4)
                    for st in range(NT):
                        xs = stg.tile([P, DM], F32, name=f"xs_{nm}{st}", tag="xs")
                        nc.sync.dma_start(out=xs, in_=x_d[st * P : (st + 1) * P, :])
                        claim(xs)
                        for dt in range(NT):
                            pt = ptr.tile(
                                [P, P], F32, name=f"pt_{nm}{st}_{dt}", tag="pt"
                            )
                            nc.tensor.transpose(
                                pt, xs[:, dt * P : (dt + 1) * P], ident
                            )
                            nc.vector.tensor_copy(
                                xt[dt][:, st * P : (st + 1) * P], pt
                            )
                return xt

            def load_w(w_d, nm, wpool):
                ws = []
                for dt in range(NT):
                    w = wpool.tile([P, DM], MM_DT, name=f"{nm}{dt}", tag=f"W{dt}")
                    nc.sync.dma_start(out=w, in_=w_d[dt * P : (dt + 1) * P, :])
                    claim(w)
                    ws.append(w)
                return ws

            def project_t(w_sb, x_t, bias_t, nm):
                """yT[do] [128, S] = (W.T @ X.T + b)[do*128:(do+1)*128, :]."""
                yt = [
                    qkv.tile([P, S], MM_DT, name=f"{nm}{d}", tag=f"{nm}{d}")
                    for d in range(NT)
                ]
                with tc.tile_pool(name=f"pp_{nm}", bufs=4, space="PSUM") as pp:
                    warm_psum(pp, [P, HALF], "ps", 4)
                    for do in range(NT):
                        for ch in range(2):
                            ps = pp.tile(
                                [P, HALF], F32, name=f"ps_{nm}{do}_{ch}", tag="ps"
                            )
                            for dt in range(NT):
                                nc.tensor.matmul(
                                    ps,
                                    (w_sb[dt][:, do * P : (do + 1) * P]),
                                    (x_t[dt][:, ch * HALF : (ch + 1) * HALF]),
                                    start=(dt == 0),
                                    stop=(dt == NT - 1),
                                )
                            nc.vector.tensor_scalar_add(
                                yt[do][:, ch * HALF : (ch + 1) * HALF],
                                ps,
                                bias_t[:, do : do + 1],
                            )
                return yt

            def project_nat(w_sb, x_t, bias_bc, nm):
                """y[st] [128, DM] = (X @ W + b)[st*128:(st+1)*128, :]."""
                y = [
                    qkv.tile([P, DM], MM_DT, name=f"{nm}{s}", tag=f"{nm}{s}")
                    for s in range(NT)
                ]
                with tc.tile_pool(name=f"pp_{nm}", bufs=4, space="PSUM") as pp:
                    warm_psum(pp, [P, HALF], "ps", 4)
                    for st in range(NT):
                        for ch in range(2):
                            ps = pp.tile(
                                [P, HALF], F32, name=f"ps_{nm}{st}_{ch}", tag="ps"
                            )
                            for dt in range(NT):
                                nc.tensor.matmul(
                                    ps,
                                    (x_t[dt][:, st * P : (st + 1) * P]),
                                    (w_sb[dt][:, ch * HALF : (ch + 1) * HALF]),
                                    start=(dt == 0),
                                    stop=(dt == NT - 1),
                                )
                            nc.vector.tensor_tensor(
                                y[st][:, ch * HALF : (ch + 1) * HALF],
                                ps,
                                bias_bc[:, ch * HALF : (ch + 1) * HALF],
                                AOP.add,
                            )
                return y

            # ---- projections (one tensor at a time to bound SBUF) ----
            with tc.tile_pool(name="wpool", bufs=1) as wpool:
                with tc.tile_pool(name="vtp", bufs=1) as vtp:
                    vt_in = load_transposed(v_d, "V", vtp)
                    wv_sb = load_w(wv_d, "wv", wpool)
                    v_sb = project_nat(wv_sb, vt_in, bv_bc, "v")

                with tc.tile_pool(name="qtp", bufs=1) as qtp:
                    qt_in = load_transposed(q_d, "Q", qtp)
                    wq_sb = load_w(wq_d, "wq", wpool)
                    q_t = project_t(wq_sb, qt_in, bqt, "qT")

                with tc.tile_pool(name="ktp", bufs=1) as ktp:
                    kt_in = load_transposed(k_d, "K", ktp)
                    wk_sb = load_w(wk_d, "wk", wpool)
                    k_t = project_t(wk_sb, kt_in, bkt, "kT")

            # attT[hp] [128, S]: rows 0:64 head 2hp, 64:128 head 2hp+1 (= concat_att.T)
            attp_cm = tc.tile_pool(name="attp", bufs=1)
            attp = attp_cm.__enter__()
            att_t = [
                attp.tile([P, S], MM_DT, name=f"attT{i}", tag=f"attT{i}")
                for i in range(NT)
            ]

            # ---- attention per head-pair ----
            with (
                tc.tile_pool(name="expp", bufs=3) as expp,
                tc.tile_pool(name="wtsp", bufs=3) as wtsp,
                tc.tile_pool(name="extp", bufs=4) as extp,
                tc.tile_pool(name="smallp", bufs=4) as smallp,
                tc.tile_pool(name="rbcp", bufs=2) as rbcp,
                tc.tile_pool(name="dscr", bufs=2, space="DRAM") as dscr,
                tc.tile_pool(name="pbig", bufs=2, space="PSUM") as pbig,
                tc.tile_pool(name="patt", bufs=1, space="PSUM") as pattp,
            ):
                warm_psum(pbig, [P, S], "pn", 2)
                for i in range(2):
                    for j in range(2):
                        warm_psum(pattp, [P, HALF], f"patt{i}{j}", 1)
                for hp in range(NT):
                    heads = (2 * hp, 2 * hp + 1)
                    recips = [
                        smallp.tile(
                            [P, NT], F32, name=f"rec{hp}_{i}", tag=f"rec{i}", bufs=2
                        )
                        for i in range(2)
                    ]

                    # natural scores -> softmax -> weights out
                    for sqt in range(NT):
                        for i, h in enumerate(heads):
                            ps = pbig.tile(
                                [P, S], F32, name=f"pn{hp}_{sqt}_{i}", tag="pn"
                            )
                            lo = 64 * i
                            for ch in range(2):
                                nc.tensor.matmul(
                                    ps[:, ch * HALF : (ch + 1) * HALF],
                                    (q_t[hp][lo : lo + 64, sqt * P : (sqt + 1) * P]),
                                    (k_t[hp][lo : lo + 64, ch * HALF : (ch + 1) * HALF]),
                                    start=True,
                                    stop=True,
                                    tile_position=(lo, 0),
                                )
                            ex = expp.tile([P, S], F32, name=f"ex{hp}_{sqt}_{i}", tag="ex")
                            sums = smallp.tile(
                                [P, 1], F32, name=f"sum{hp}_{sqt}_{i}", tag="sums"
                            )
                            nc.scalar.activation(
                                ex, ps, ACT_EXP, scale=0.125, accum_out=sums
                            )
                            rc = recips[i][:, sqt : sqt + 1]
                            nc.vector.reciprocal(rc, sums)
                            wt = wtsp.tile(
                                [P, S], F32, name=f"wt{hp}_{sqt}_{i}", tag="wt"
                            )
                            nc.vector.tensor_scalar_mul(wt, ex, rc)
                            nc.sync.dma_start(
                                out=wts_d[h, sqt * P : (sqt + 1) * P, :], in_=wt
                            )

                    # reciprocal rows for attT normalization: [128, 8] -> [8, 128]
                    # -> DRAM -> partition-broadcast [64, S] per head
                    rbc = rbcp.tile([P, S], F32, name=f"rbc{hp}", tag="rbc")
                    for i, h in enumerate(heads):
                        prt = pbig.tile([NT, P], F32, name=f"prt{hp}_{i}", tag="pn")
                        nc.tensor.transpose(prt, recips[i], ident)
                        r_t = smallp.tile(
                            [NT, P], F32, name=f"rT{hp}_{i}", tag="rT", bufs=3
                        )
                        nc.vector.tensor_copy(r_t, prt)
                        scr = dscr.tile([NT, P], F32, name=f"scr{hp}_{i}", tag="scr")
                        nc.sync.dma_start(out=scr, in_=r_t)
                        nc.sync.dma_start(
                            out=rbc[64 * i : 64 * (i + 1), :],
                            in_=_bcast(scr.rearrange("a b -> (a b)"), 64),
                        )

                    # transposed scores -> exp -> attT accumulation.
                    # f32r matmuls require M=128, so each head's matmul uses the
                    # full v pair-block [128k, 128dm] as lhsT; only rows
                    # 64i:64i+64 of its psum hold that head's attT rows.
                    patt = [
                        pattp.tile(
                            [P, HALF], F32, name=f"patt{hp}_{i}_{j}", tag=f"patt{i}{j}"
                        )
                        for i in range(2)
                        for j in range(2)
                    ]  # index 2*i + half (head i)
                    for kt in range(NT):
                        exts = []
                        for i, h in enumerate(heads):
                            lo = 64 * i
                            pst = pbig.tile([P, S], F32, name=f"pt{hp}_{kt}_{i}", tag="pn")
                            for ch in range(2):
                                nc.tensor.matmul(
                                    pst[:, ch * HALF : (ch + 1) * HALF],
                                    (k_t[hp][lo : lo + 64, kt * P : (kt + 1) * P]),
                                    (q_t[hp][lo : lo + 64, ch * HALF : (ch + 1) * HALF]),
                                    start=True,
                                    stop=True,
                                    tile_position=(lo, 0),
                                )
                            ext = extp.tile([P, S], MM_DT, name=f"ext{hp}_{kt}_{i}", tag="ext")
                            nc.scalar.activation(ext, pst, ACT_EXP, scale=0.125)
                            exts.append(ext)
                        for i in range(2):
                            for half in range(2):
                                nc.tensor.matmul(
                                    patt[2 * i + half],
                                    v_sb[kt][:, P * hp : P * (hp + 1)],
                                    (exts[i][:, half * HALF : (half + 1) * HALF]),
                                    start=(kt == 0),
                                    stop=(kt == NT - 1),
                                )
                    for i in range(2):
                        lo = 64 * i
                        for half in range(2):
                            nc.vector.tensor_tensor(
                                att_t[hp][lo : lo + 64, half * HALF : (half + 1) * HALF],
                                patt[2 * i + half][lo : lo + 64, :],
                                rbc[lo : lo + 64, half * HALF : (half + 1) * HALF],
                                AOP.mult,
                            )

            # ---- output projection ----
            with (
                tc.tile_pool(name="wop", bufs=1) as wop,
                tc.tile_pool(name="pout", bufs=4, space="PSUM") as pout,
                tc.tile_pool(name="outp", bufs=4) as outp,
            ):
                warm_psum(pout, [P, HALF], "po", 4)
                wo_sb = load_w(wo_d, "wo", wop)
                for st in range(NT):
                    for ch in range(2):
                        po = pout.tile([P, HALF], F32, name=f"po{st}_{ch}", tag="po")
                        for kt in range(NT):
                            nc.tensor.matmul(
                                po,
                                (att_t[kt][:, st * P : (st + 1) * P]),
                                (wo_sb[kt][:, ch * HALF : (ch + 1) * HALF]),
                                start=(kt == 0),
                                stop=(kt == NT - 1),
                            )
                        ob = outp.tile([P, HALF], F32, name=f"ob{st}_{ch}", tag="ob")
                        nc.vector.tensor_tensor(
                            ob, po, bo_bc[:, ch * HALF : (ch + 1) * HALF], AOP.add
                        )
                        nc.sync.dma_start(
                            out=out_d[st * P : (st + 1) * P, ch * HALF : (ch + 1) * HALF],
                            in_=ob,
                        )
            attp_cm.__exit__(None, None, None)

    nc.compile()
    return nc


_CACHE = {}


def _get_nc():
    if "nc" not in _CACHE:
        _CACHE["nc"] = build_nc()
    return _CACHE["nc"]


def _make_in_maps(inputs):
    f32 = lambda x: np.ascontiguousarray(np.asarray(x, dtype=np.float32))
    shared = {
        n: f32(inputs[n])
        for n in ("Wq", "bq", "Wk", "bk", "Wv", "bv", "Wo", "bo")
    }
    q, k, v = (np.asarray(inputs[n], dtype=np.float32) for n in ("Q", "K", "V"))
    shared["ident"] = np.eye(P, dtype=np.float32)
    return [
        {"Qb": f32(q[b]), "Kb": f32(k[b]), "Vb": f32(v[b]), **shared}
        for b in range(B)
    ]


def run(inputs, **spmd_kwargs):
    res = run_bass_kernel_spmd(
        _get_nc(), _make_in_maps(inputs), core_ids=list(range(B)), **spmd_kwargs
    )
    out = np.stack([r["out"] for r in res.results], axis=0)
    wts = np.stack([r["wts"] for r in res.results], axis=0)
    return (out, wts), res


def kernel(**inputs):
    (out, wts), _ = run(inputs)
    return out, wts
